# revision 9
# baseline (speedup 1.0000x reference)
"""CISSVAE (cluster-routed VAE) Trainium2 kernel.

Strategy: expert-parallel over the 8 clusters — core c handles exactly the rows
with cluster_labels == c (capacity-padded to a fixed CAP so all 8 cores run one
SPMD program). Host does the routing (gather by cluster, pad, transpose to
feature-major) and the inverse scatter. On-device everything is dense matmuls:

    h0 = relu(W_enc0[c]^T x + b)   [1024, CAP]
    h1 = relu(W_enc1^T h0 + b)     [512, CAP]
    lv = W_logvar^T h1 + b         [128, CAP]
    mu = W_mu^T h1 + b             [128, CAP]
    z  = mu + eps * exp(0.5 lv)    [128, CAP]
    d0 = relu(W_dec0^T z + b)      [512, CAP]
    d1 = relu(W_dec1[c]^T d0 + b)  [1024, CAP]
    recon = W_out^T d1 + b         [2048, CAP]

Activations live feature-major ([feature, column]) so every layer is
out[o, n] += W[k, o] * act[k, n] with W slices as the stationary operand —
no transposes anywhere on device. Matmuls run in float32r (TF32-like, full
PE rate for moving dim >= 256), accumulation in fp32 PSUM, bias+activation
fused into the ScalarE PSUM evacuation. All weights are host-packed into
[OT, 128, KT, 128] per-o-tile blocks so every weight DMA is contiguous.
"""

import math
from contextlib import ExitStack

import numpy as np

import concourse.bacc as bacc
import concourse.mybir as mybir
import concourse.tile as tile
from concourse.bass_utils import run_bass_kernel_spmd

F32 = mybir.dt.float32
F32R = mybir.dt.float32r
AF = mybir.ActivationFunctionType

D_IN, H0, H1, LAT, C = 2048, 1024, 512, 128, 8
N_CORES = 8
P = 128

# (name, K, O) for the seven dense layers, in execution order
LAYERS = [
    ("enc0", D_IN, H0),
    ("enc1", H0, H1),
    ("lv", H1, LAT),
    ("mu", H1, LAT),
    ("dec0", LAT, H1),
    ("dec1", H1, H0),
    ("out", H0, D_IN),
]
BIAS_COLS = sum(o // P for _, _, o in LAYERS)  # 42

_program_cache: dict = {}


def _nchunks(cap):
    """Split cap columns into balanced chunks, each in [256, 512] and a
    multiple of 4 (fp32r matmul ISA restriction on moving dim/offset)."""
    assert cap % 4 == 0
    k = max(1, math.ceil(cap / 512))
    if k > 1 and cap / k < 256:
        k -= 1
    q = cap // 4
    base = q // k
    rem = q - base * k
    sizes = [4 * (base + (1 if i < rem else 0)) for i in range(k)]
    assert all(256 <= s <= 512 for s in sizes) or cap < 256, (cap, sizes)
    out, acc = [], 0
    for s in sizes:
        out.append((acc, s))
        acc += s
    return out


def _build(cap):
    nbs = _nchunks(cap)
    nc = bacc.Bacc(trn_type="TRN2", target_bir_lowering=False, debug=False)

    KT_X = D_IN // P
    xh = nc.dram_tensor("xh", [P, KT_X, cap], F32R, kind="ExternalInput")
    epsT = nc.dram_tensor("epsT", [LAT, cap], F32, kind="ExternalInput")
    w_d = {
        name: nc.dram_tensor(f"w_{name}", [o // P, P, k // P, P], F32R,
                             kind="ExternalInput")
        for name, k, o in LAYERS
    }
    bias_d = nc.dram_tensor("bias_all", [P, BIAS_COLS], F32, kind="ExternalInput")

    reconT = nc.dram_tensor("reconT", [D_IN, cap], F32, kind="ExternalOutput")
    muT = nc.dram_tensor("muT", [LAT, cap], F32, kind="ExternalOutput")
    lvT = nc.dram_tensor("lvT", [LAT, cap], F32, kind="ExternalOutput")

    bias_off = {}
    acc = 0
    for name, _, o in LAYERS:
        bias_off[name] = acc
        acc += o // P

    with tile.TileContext(nc) as tc, ExitStack() as ctx:
        data = ctx.enter_context(tc.tile_pool(name="data", bufs=1))
        wstream = ctx.enter_context(tc.tile_pool(name="wstream", bufs=4))
        stage = ctx.enter_context(tc.tile_pool(name="stage", bufs=4))
        psum = ctx.enter_context(tc.tile_pool(name="psum", bufs=8, space="PSUM"))

        # x: split per (n-chunk, k-tile) so enc0's first PSUM groups can start
        # as soon as the first column-chunk lands instead of after all 8.5MB
        xt = data.tile([P, KT_X, cap], F32R, tag="slabA")
        for n0, nb in nbs:
            for k in range(KT_X):
                nc.sync.dma_start(
                    xt[:, k, n0 : n0 + nb], xh[:, k, n0 : n0 + nb]
                )

        bias_sb = data.tile([P, BIAS_COLS], F32)
        nc.sync.dma_start(bias_sb[:], bias_d[:, :])

        def dense(inp, lname, act, out_dt, out_tag=None, evac=None):
            """One dense layer; weights streamed per 128-wide o-slice from the
            packed DRAM block. inp: SBUF [P, KT, cap]. Returns [P, OT, cap]
            (unless evac is given, which handles PSUM evacuation itself)."""
            _, K, O = next(l for l in LAYERS if l[0] == lname)
            KT, OT = K // P, O // P
            out = None
            if evac is None:
                out = data.tile(
                    [P, OT, cap], out_dt, name=f"a_{lname}", tag=out_tag or f"a_{lname}"
                )
            for o in range(OT):
                wt = wstream.tile([P, 16, P], F32R, tag="wstream", name=f"{lname}_w{o}")
                nc.sync.dma_start(wt[:, :KT, :], w_d[lname][o, :, :, :])
                for n0, nb in nbs:
                    ps = psum.tile([P, 512], F32, tag="ps", name=f"{lname}_ps{o}")
                    for k in range(KT):
                        nc.tensor.matmul(
                            ps[:, :nb],
                            wt[:, k, :],
                            inp[:, k, n0 : n0 + nb],
                            start=(k == 0),
                            stop=(k == KT - 1),
                        )
                    b_ap = bias_sb[:, bias_off[lname] + o : bias_off[lname] + o + 1]
                    if evac is None:
                        nc.scalar.activation(
                            out[:, o, n0 : n0 + nb], ps[:, :nb], act, bias=b_ap
                        )
                    else:
                        evac(o, n0, nb, ps, b_ap)
            return out

        # encoder
        h0 = dense(xt, "enc0", AF.Relu, F32R)
        h1 = dense(h0, "enc1", AF.Relu, F32R)

        # latent heads: lv first so the exp/z chain overlaps the mu matmuls
        lv_sb = dense(h1, "lv", AF.Identity, F32)
        t_sb = data.tile([P, cap], F32)
        nc.scalar.activation(t_sb[:], lv_sb[:, 0, :], AF.Exp, scale=0.5)
        nc.sync.dma_start(lvT[:, :], lv_sb[:, 0, :])

        mu_sb = dense(h1, "mu", AF.Identity, F32)
        nc.sync.dma_start(muT[:, :], mu_sb[:, 0, :])

        # z = mu + eps * exp(0.5 lv)
        eps_sb = data.tile([P, cap], F32)
        nc.sync.dma_start(eps_sb[:], epsT[:, :])
        z_sb = data.tile([P, 1, cap], F32R)
        nc.vector.scalar_tensor_tensor(
            z_sb[:, 0, :],
            eps_sb[:],
            1.0,
            t_sb[:],
            mybir.AluOpType.mult,
            mybir.AluOpType.mult,
        )
        nc.vector.tensor_add(z_sb[:, 0, :], z_sb[:, 0, :], mu_sb[:, 0, :])

        # decoder
        d0 = dense(z_sb, "dec0", AF.Relu, F32R, out_tag="a_enc1")
        d1 = dense(d0, "dec1", AF.Relu, F32R, out_tag="a_enc0")

        # output layer: stage through SBUF, DMA out per (o, n)
        def out_evac(o, n0, nb, ps, b_ap):
            st = stage.tile([P, 512], F32, tag="ostage", name=f"out_st{o}")
            nc.scalar.activation(st[:, :nb], ps[:, :nb], AF.Identity, bias=b_ap)
            nc.sync.dma_start(reconT[o * P : (o + 1) * P, n0 : n0 + nb], st[:, :nb])

        dense(d1, "out", AF.Identity, F32, evac=out_evac)

    nc.compile()
    return nc


def _get_program(cap):
    if cap not in _program_cache:
        _program_cache[cap] = _build(cap)
    return _program_cache[cap]


def _pack_w(w):
    """[K, O] -> [OT, 128, KT, 128] contiguous per-o-tile blocks."""
    K, O = w.shape
    return np.ascontiguousarray(
        w.reshape(K // P, P, O // P, P).transpose(2, 1, 0, 3)
    )


def _pack_b(b):
    """[O] -> [128, OT]"""
    return np.ascontiguousarray(b.reshape(-1, P).T)


def _route(cluster_labels):
    labels = np.asarray(cluster_labels).astype(np.int64)
    idx = [np.nonzero(labels == c)[0] for c in range(C)]
    cap = max(256, max((len(i) for i in idx), default=1))
    cap = (cap + 3) // 4 * 4
    idx_pad = []
    for c in range(C):
        i = idx[c]
        if len(i) == 0:
            pad = np.zeros(cap, dtype=np.int64)
        else:
            pad = np.concatenate([i, np.full(cap - len(i), i[0], dtype=np.int64)])
        idx_pad.append(pad)
    return idx, idx_pad, cap


def _run(inputs, trace=False):
    x = np.ascontiguousarray(np.asarray(inputs["x"], dtype=np.float32))
    eps = np.ascontiguousarray(np.asarray(inputs["eps"], dtype=np.float32))
    idx, idx_pad, cap = _route(inputs["cluster_labels"])
    nc = _get_program(cap)

    f32 = lambda a: np.asarray(a, dtype=np.float32)
    shared_w = {
        "w_enc1": _pack_w(f32(inputs["W_enc1"])),
        "w_lv": _pack_w(f32(inputs["W_logvar"])),
        "w_mu": _pack_w(f32(inputs["W_mu"])),
        "w_dec0": _pack_w(f32(inputs["W_dec0"])),
        "w_out": _pack_w(f32(inputs["W_out"])),
    }
    b_shared = {
        "enc1": _pack_b(f32(inputs["b_enc1"])),
        "lv": _pack_b(f32(inputs["b_logvar"])),
        "mu": _pack_b(f32(inputs["b_mu"])),
        "dec0": _pack_b(f32(inputs["b_dec0"])),
        "out": _pack_b(f32(inputs["b_out"])),
    }
    in_maps = []
    for c in range(C):
        ip = idx_pad[c]
        xT = x[ip].T  # [D_IN, cap]
        bias_all = np.concatenate(
            [
                _pack_b(f32(inputs["b_enc0"][c])),
                b_shared["enc1"],
                b_shared["lv"],
                b_shared["mu"],
                b_shared["dec0"],
                _pack_b(f32(inputs["b_dec1"][c])),
                b_shared["out"],
            ],
            axis=1,
        )
        in_maps.append(
            {
                "xh": np.ascontiguousarray(
                    xT.reshape(D_IN // P, P, cap).transpose(1, 0, 2)
                ),
                "epsT": np.ascontiguousarray(eps[ip].T),
                "w_enc0": _pack_w(f32(inputs["W_enc0"][c])),
                "w_dec1": _pack_w(f32(inputs["W_dec1"][c])),
                "bias_all": np.ascontiguousarray(bias_all),
                **shared_w,
            }
        )

    res = run_bass_kernel_spmd(nc, in_maps, core_ids=list(range(N_CORES)), trace=trace)

    B = x.shape[0]
    recon = np.empty((B, D_IN), dtype=np.float32)
    mu = np.empty((B, LAT), dtype=np.float32)
    logvar = np.empty((B, LAT), dtype=np.float32)
    for c in range(C):
        i = idx[c]
        if len(i) == 0:
            continue
        r = res.results[c]
        recon[i] = r["reconT"][:, : len(i)].T
        mu[i] = r["muT"][:, : len(i)].T
        logvar[i] = r["lvT"][:, : len(i)].T
    return (recon, mu, logvar), res


def kernel(**inputs):
    outs, _ = _run(inputs, trace=False)
    return outs


# revision 10
# speedup vs baseline: 1.1998x; 1.1998x over previous
"""CISSVAE (cluster-routed VAE) Trainium2 kernel.

Strategy: expert-parallel over the 8 clusters — core c handles exactly the rows
with cluster_labels == c (capacity-padded to a fixed CAP so all 8 cores run one
SPMD program). Host does the routing (gather by cluster, pad, transpose to
feature-major) and the inverse scatter. On-device everything is dense matmuls:

    h0 = relu(W_enc0[c]^T x + b)   [1024, CAP]
    h1 = relu(W_enc1^T h0 + b)     [512, CAP]
    lv = W_logvar^T h1 + b         [128, CAP]
    mu = W_mu^T h1 + b             [128, CAP]
    z  = mu + eps * exp(0.5 lv)    [128, CAP]
    d0 = relu(W_dec0^T z + b)      [512, CAP]
    d1 = relu(W_dec1[c]^T d0 + b)  [1024, CAP]
    recon = W_out^T d1 + b         [2048, CAP]

Activations live feature-major ([feature, column]) so every layer is
out[o, n] += W[k, o] * act[k, n] with W slices as the stationary operand —
no transposes anywhere on device. Matmuls run in float32r (TF32-like, full
PE rate for moving dim >= 256), accumulation in fp32 PSUM, bias+activation
fused into the ScalarE PSUM evacuation. All weights are host-packed into
[OT, 128, KT, 128] per-o-tile blocks so every weight DMA is contiguous.
"""

import math
from contextlib import ExitStack

import numpy as np

import concourse.bacc as bacc
import concourse.mybir as mybir
import concourse.tile as tile
from concourse.bass_utils import run_bass_kernel_spmd

F32 = mybir.dt.float32
F32R = mybir.dt.float32r
AF = mybir.ActivationFunctionType

D_IN, H0, H1, LAT, C = 2048, 1024, 512, 128, 8
N_CORES = 8
P = 128

# (name, K, O) for the seven dense layers, in execution order
LAYERS = [
    ("enc0", D_IN, H0),
    ("enc1", H0, H1),
    ("lv", H1, LAT),
    ("mu", H1, LAT),
    ("dec0", LAT, H1),
    ("dec1", H1, H0),
    ("out", H0, D_IN),
]
BIAS_COLS = sum(o // P for _, _, o in LAYERS)  # 42

_program_cache: dict = {}


def _nchunks(cap):
    """Split cap columns into balanced chunks, each in [256, 512] and a
    multiple of 4 (fp32r matmul ISA restriction on moving dim/offset)."""
    assert cap % 4 == 0
    k = max(1, math.ceil(cap / 512))
    if k > 1 and cap / k < 256:
        k -= 1
    q = cap // 4
    base = q // k
    rem = q - base * k
    sizes = [4 * (base + (1 if i < rem else 0)) for i in range(k)]
    assert all(256 <= s <= 512 for s in sizes) or cap < 256, (cap, sizes)
    out, acc = [], 0
    for s in sizes:
        out.append((acc, s))
        acc += s
    return out


def _build(cap):
    nbs = _nchunks(cap)
    nc = bacc.Bacc(trn_type="TRN2", target_bir_lowering=False, debug=False)

    KT_X = D_IN // P
    xh = nc.dram_tensor("xh", [P, KT_X, cap], F32R, kind="ExternalInput")
    epsT = nc.dram_tensor("epsT", [LAT, cap], F32, kind="ExternalInput")
    w_d = {
        name: nc.dram_tensor(f"w_{name}", [o // P, P, k // P, P], F32R,
                             kind="ExternalInput")
        for name, k, o in LAYERS
    }
    bias_d = nc.dram_tensor("bias_all", [P, BIAS_COLS], F32, kind="ExternalInput")

    reconT = nc.dram_tensor("reconT", [D_IN, cap], F32, kind="ExternalOutput")
    muT = nc.dram_tensor("muT", [LAT, cap], F32, kind="ExternalOutput")
    lvT = nc.dram_tensor("lvT", [LAT, cap], F32, kind="ExternalOutput")

    bias_off = {}
    acc = 0
    for name, _, o in LAYERS:
        bias_off[name] = acc
        acc += o // P

    with tile.TileContext(nc) as tc, ExitStack() as ctx:
        data = ctx.enter_context(tc.tile_pool(name="data", bufs=1))
        wstream = ctx.enter_context(tc.tile_pool(name="wstream", bufs=4))
        stage = ctx.enter_context(tc.tile_pool(name="stage", bufs=4))
        psum = ctx.enter_context(tc.tile_pool(name="psum", bufs=8, space="PSUM"))

        # x: per-k-tile contiguous DMAs (parallel queues, fine-grained deps)
        xt = data.tile([P, KT_X, cap], F32R, tag="slabA")
        for k in range(KT_X):
            nc.sync.dma_start(xt[:, k, :], xh[:, k, :])

        bias_sb = data.tile([P, BIAS_COLS], F32)
        nc.sync.dma_start(bias_sb[:], bias_d[:, :])

        def dense(inp, lname, act, out_dt, out_tag=None, evac=None):
            """One dense layer; weights streamed per 128-wide o-slice from the
            packed DRAM block. inp: SBUF [P, KT, cap]. Returns [P, OT, cap]
            (unless evac is given, which handles PSUM evacuation itself)."""
            _, K, O = next(l for l in LAYERS if l[0] == lname)
            KT, OT = K // P, O // P
            out = None
            if evac is None:
                out = data.tile(
                    [P, OT, cap], out_dt, name=f"a_{lname}", tag=out_tag or f"a_{lname}"
                )
            for o in range(OT):
                wt = wstream.tile([P, 16, P], F32R, tag="wstream", name=f"{lname}_w{o}")
                nc.sync.dma_start(wt[:, :KT, :], w_d[lname][o, :, :, :])
                for n0, nb in nbs:
                    ps = psum.tile([P, 512], F32, tag="ps", name=f"{lname}_ps{o}")
                    for k in range(KT):
                        nc.tensor.matmul(
                            ps[:, :nb],
                            wt[:, k, :],
                            inp[:, k, n0 : n0 + nb],
                            start=(k == 0),
                            stop=(k == KT - 1),
                        )
                    b_ap = bias_sb[:, bias_off[lname] + o : bias_off[lname] + o + 1]
                    if evac is None:
                        nc.scalar.activation(
                            out[:, o, n0 : n0 + nb], ps[:, :nb], act, bias=b_ap
                        )
                    else:
                        evac(o, n0, nb, ps, b_ap)
            return out

        # encoder
        h0 = dense(xt, "enc0", AF.Relu, F32R)
        h1 = dense(h0, "enc1", AF.Relu, F32R)

        # latent heads: lv first so the exp/z chain overlaps the mu matmuls
        lv_sb = dense(h1, "lv", AF.Identity, F32)
        t_sb = data.tile([P, cap], F32)
        nc.scalar.activation(t_sb[:], lv_sb[:, 0, :], AF.Exp, scale=0.5)
        nc.sync.dma_start(lvT[:, :], lv_sb[:, 0, :])

        mu_sb = dense(h1, "mu", AF.Identity, F32)
        nc.sync.dma_start(muT[:, :], mu_sb[:, 0, :])

        # z = mu + eps * exp(0.5 lv)
        eps_sb = data.tile([P, cap], F32)
        nc.sync.dma_start(eps_sb[:], epsT[:, :])
        z_sb = data.tile([P, 1, cap], F32R)
        nc.vector.scalar_tensor_tensor(
            z_sb[:, 0, :],
            eps_sb[:],
            1.0,
            t_sb[:],
            mybir.AluOpType.mult,
            mybir.AluOpType.mult,
        )
        nc.vector.tensor_add(z_sb[:, 0, :], z_sb[:, 0, :], mu_sb[:, 0, :])

        # decoder
        d0 = dense(z_sb, "dec0", AF.Relu, F32R, out_tag="a_enc1")
        d1 = dense(d0, "dec1", AF.Relu, F32R, out_tag="a_enc0")

        # output layer: stage through SBUF, DMA out per (o, n)
        def out_evac(o, n0, nb, ps, b_ap):
            st = stage.tile([P, 512], F32, tag="ostage", name=f"out_st{o}")
            nc.scalar.activation(st[:, :nb], ps[:, :nb], AF.Identity, bias=b_ap)
            nc.sync.dma_start(reconT[o * P : (o + 1) * P, n0 : n0 + nb], st[:, :nb])

        dense(d1, "out", AF.Identity, F32, evac=out_evac)

    nc.compile()
    return nc


def _get_program(cap):
    if cap not in _program_cache:
        _program_cache[cap] = _build(cap)
    return _program_cache[cap]


def _pack_w(w):
    """[K, O] -> [OT, 128, KT, 128] contiguous per-o-tile blocks."""
    K, O = w.shape
    return np.ascontiguousarray(
        w.reshape(K // P, P, O // P, P).transpose(2, 1, 0, 3)
    )


def _pack_b(b):
    """[O] -> [128, OT]"""
    return np.ascontiguousarray(b.reshape(-1, P).T)


def _route(cluster_labels):
    labels = np.asarray(cluster_labels).astype(np.int64)
    idx = [np.nonzero(labels == c)[0] for c in range(C)]
    cap = max(256, max((len(i) for i in idx), default=1))
    cap = (cap + 3) // 4 * 4
    idx_pad = []
    for c in range(C):
        i = idx[c]
        if len(i) == 0:
            pad = np.zeros(cap, dtype=np.int64)
        else:
            pad = np.concatenate([i, np.full(cap - len(i), i[0], dtype=np.int64)])
        idx_pad.append(pad)
    return idx, idx_pad, cap


def _run(inputs, trace=False):
    x = np.ascontiguousarray(np.asarray(inputs["x"], dtype=np.float32))
    eps = np.ascontiguousarray(np.asarray(inputs["eps"], dtype=np.float32))
    idx, idx_pad, cap = _route(inputs["cluster_labels"])
    nc = _get_program(cap)

    f32 = lambda a: np.asarray(a, dtype=np.float32)
    shared_w = {
        "w_enc1": _pack_w(f32(inputs["W_enc1"])),
        "w_lv": _pack_w(f32(inputs["W_logvar"])),
        "w_mu": _pack_w(f32(inputs["W_mu"])),
        "w_dec0": _pack_w(f32(inputs["W_dec0"])),
        "w_out": _pack_w(f32(inputs["W_out"])),
    }
    b_shared = {
        "enc1": _pack_b(f32(inputs["b_enc1"])),
        "lv": _pack_b(f32(inputs["b_logvar"])),
        "mu": _pack_b(f32(inputs["b_mu"])),
        "dec0": _pack_b(f32(inputs["b_dec0"])),
        "out": _pack_b(f32(inputs["b_out"])),
    }
    in_maps = []
    for c in range(C):
        ip = idx_pad[c]
        xT = x[ip].T  # [D_IN, cap]
        bias_all = np.concatenate(
            [
                _pack_b(f32(inputs["b_enc0"][c])),
                b_shared["enc1"],
                b_shared["lv"],
                b_shared["mu"],
                b_shared["dec0"],
                _pack_b(f32(inputs["b_dec1"][c])),
                b_shared["out"],
            ],
            axis=1,
        )
        in_maps.append(
            {
                "xh": np.ascontiguousarray(
                    xT.reshape(D_IN // P, P, cap).transpose(1, 0, 2)
                ),
                "epsT": np.ascontiguousarray(eps[ip].T),
                "w_enc0": _pack_w(f32(inputs["W_enc0"][c])),
                "w_dec1": _pack_w(f32(inputs["W_dec1"][c])),
                "bias_all": np.ascontiguousarray(bias_all),
                **shared_w,
            }
        )

    res = run_bass_kernel_spmd(nc, in_maps, core_ids=list(range(N_CORES)), trace=trace)

    B = x.shape[0]
    recon = np.empty((B, D_IN), dtype=np.float32)
    mu = np.empty((B, LAT), dtype=np.float32)
    logvar = np.empty((B, LAT), dtype=np.float32)
    for c in range(C):
        i = idx[c]
        if len(i) == 0:
            continue
        r = res.results[c]
        recon[i] = r["reconT"][:, : len(i)].T
        mu[i] = r["muT"][:, : len(i)].T
        logvar[i] = r["lvT"][:, : len(i)].T
    return (recon, mu, logvar), res


def kernel(**inputs):
    outs, _ = _run(inputs, trace=False)
    return outs


# revision 15
# speedup vs baseline: 1.2905x; 1.0756x over previous
"""CISSVAE (cluster-routed VAE) Trainium2 kernel.

Strategy: expert-parallel over the 8 clusters — core c handles exactly the rows
with cluster_labels == c (capacity-padded to a fixed CAP so all 8 cores run one
SPMD program). Host does the routing (gather by cluster, pad, transpose to
feature-major) and the inverse scatter. On-device everything is dense matmuls:

    h0 = relu(W_enc0[c]^T x + b)   [1024, CAP]
    h1 = relu(W_enc1^T h0 + b)     [512, CAP]
    lv = W_logvar^T h1 + b         [128, CAP]
    mu = W_mu^T h1 + b             [128, CAP]
    z  = mu + eps * exp(0.5 lv)    [128, CAP]
    d0 = relu(W_dec0^T z + b)      [512, CAP]
    d1 = relu(W_dec1[c]^T d0 + b)  [1024, CAP]
    recon = W_out^T d1 + b         [2048, CAP]

Activations live feature-major ([feature, column]) so every layer is
out[o, n] += W[k, o] * act[k, n] with W slices as the stationary operand —
no transposes anywhere on device. Matmuls run in float32r (TF32-like, full
PE rate for moving dim >= 256), accumulation in fp32 PSUM, bias+activation
fused into the ScalarE PSUM evacuation. All weights are host-packed into
[OT, 128, KT, 128] per-o-tile blocks so every weight DMA is contiguous.
"""

import math
from contextlib import ExitStack

import numpy as np

import concourse.bacc as bacc
import concourse.mybir as mybir
import concourse.tile as tile
from concourse.bass_utils import run_bass_kernel_spmd

F32 = mybir.dt.float32
F32R = mybir.dt.float32r
AF = mybir.ActivationFunctionType

D_IN, H0, H1, LAT, C = 2048, 1024, 512, 128, 8
N_CORES = 8
P = 128

# (name, K, O) for the seven dense layers, in execution order
LAYERS = [
    ("enc0", D_IN, H0),
    ("enc1", H0, H1),
    ("lv", H1, LAT),
    ("mu", H1, LAT),
    ("dec0", LAT, H1),
    ("dec1", H1, H0),
    ("out", H0, D_IN),
]
BIAS_COLS = sum(o // P for _, _, o in LAYERS)  # 42

_program_cache: dict = {}


def _nchunks(cap):
    """Split cap columns into balanced chunks, each in [256, 512] and a
    multiple of 4 (fp32r matmul ISA restriction on moving dim/offset)."""
    assert cap % 4 == 0
    k = max(1, math.ceil(cap / 512))
    if k > 1 and cap / k < 256:
        k -= 1
    q = cap // 4
    base = q // k
    rem = q - base * k
    sizes = [4 * (base + (1 if i < rem else 0)) for i in range(k)]
    assert all(256 <= s <= 512 for s in sizes) or cap < 256, (cap, sizes)
    out, acc = [], 0
    for s in sizes:
        out.append((acc, s))
        acc += s
    return out


def _build(cap):
    nbs = _nchunks(cap)
    nc = bacc.Bacc(trn_type="TRN2", target_bir_lowering=False, debug=False)

    KT_X = D_IN // P
    xh = nc.dram_tensor("xh", [P, KT_X, cap], F32R, kind="ExternalInput")
    epsT = nc.dram_tensor("epsT", [LAT, cap], F32, kind="ExternalInput")
    w_d = {
        name: nc.dram_tensor(f"w_{name}", [o // P, P, k // P, P], F32R,
                             kind="ExternalInput")
        for name, k, o in LAYERS
    }
    bias_d = nc.dram_tensor("bias_all", [P, BIAS_COLS], F32, kind="ExternalInput")

    reconT = nc.dram_tensor("reconT", [D_IN, cap], F32, kind="ExternalOutput")
    muT = nc.dram_tensor("muT", [LAT, cap], F32, kind="ExternalOutput")
    lvT = nc.dram_tensor("lvT", [LAT, cap], F32, kind="ExternalOutput")

    bias_off = {}
    acc = 0
    for name, _, o in LAYERS:
        bias_off[name] = acc
        acc += o // P

    with tile.TileContext(nc) as tc, ExitStack() as ctx:
        data = ctx.enter_context(tc.tile_pool(name="data", bufs=1))
        wstream = ctx.enter_context(tc.tile_pool(name="wstream", bufs=4))
        stage = ctx.enter_context(tc.tile_pool(name="stage", bufs=4))
        psum = ctx.enter_context(tc.tile_pool(name="psum", bufs=8, space="PSUM"))

        # enc0's first weight slice goes first so it heads its DMA queue:
        # the very first matmul then only waits on it + x[0]
        w_enc0_0 = wstream.tile([P, 16, P], F32R, tag="wstream", name="enc0_w0pre")
        nc.sync.dma_start(w_enc0_0[:, :, :], w_d["enc0"][0, :, :, :])

        # x: per-k-tile contiguous DMAs (parallel queues, fine-grained deps)
        xt = data.tile([P, KT_X, cap], F32R, tag="slabA")
        for k in range(KT_X):
            nc.sync.dma_start(xt[:, k, :], xh[:, k, :])

        bias_sb = data.tile([P, BIAS_COLS], F32)
        nc.sync.dma_start(bias_sb[:], bias_d[:, :])

        # small mid-layer weights resident, prefetched up front so the
        # lv->mu->z->dec0 section never waits on DMA
        def load_resident(lname):
            _, K, O = next(l for l in LAYERS if l[0] == lname)
            KT = K // P
            wt = data.tile([P, KT, O // P, P], F32R, name=f"wres_{lname}")
            nc.sync.dma_start(
                wt[:], w_d[lname].rearrange("ot kp kt of -> kp kt ot of")
            )
            return wt

        w_res = {n: load_resident(n) for n in ("lv", "mu", "dec0")}

        def dense(inp, lname, act, out_dt, out_tag=None, evac=None):
            """One dense layer; weights streamed per 128-wide o-slice from the
            packed DRAM block. inp: SBUF [P, KT, cap]. Returns [P, OT, cap]
            (unless evac is given, which handles PSUM evacuation itself)."""
            _, K, O = next(l for l in LAYERS if l[0] == lname)
            KT, OT = K // P, O // P
            out = None
            if evac is None:
                out = data.tile(
                    [P, OT, cap], out_dt, name=f"a_{lname}", tag=out_tag or f"a_{lname}"
                )
            for o in range(OT):
                if lname in w_res:
                    w_ap = lambda k, o=o: w_res[lname][:, k, o, :]
                elif lname == "enc0" and o == 0:
                    w_ap = lambda k: w_enc0_0[:, k, :]
                else:
                    wt = wstream.tile(
                        [P, 16, P], F32R, tag="wstream", name=f"{lname}_w{o}"
                    )
                    nc.sync.dma_start(wt[:, :KT, :], w_d[lname][o, :, :, :])
                    w_ap = lambda k, wt=wt: wt[:, k, :]
                for n0, nb in nbs:
                    ps = psum.tile([P, 512], F32, tag="ps", name=f"{lname}_ps{o}")
                    for k in range(KT):
                        nc.tensor.matmul(
                            ps[:, :nb],
                            w_ap(k),
                            inp[:, k, n0 : n0 + nb],
                            start=(k == 0),
                            stop=(k == KT - 1),
                        )
                    b_ap = bias_sb[:, bias_off[lname] + o : bias_off[lname] + o + 1]
                    if evac is None:
                        nc.scalar.activation(
                            out[:, o, n0 : n0 + nb], ps[:, :nb], act, bias=b_ap
                        )
                    else:
                        evac(o, n0, nb, ps, b_ap)
            return out

        # encoder
        h0 = dense(xt, "enc0", AF.Relu, F32R)
        h1 = dense(h0, "enc1", AF.Relu, F32R)

        # latent heads: lv first so the exp/z chain overlaps the mu matmuls
        lv_sb = dense(h1, "lv", AF.Identity, F32)
        t_sb = data.tile([P, cap], F32)
        nc.scalar.activation(t_sb[:], lv_sb[:, 0, :], AF.Exp, scale=0.5)
        nc.sync.dma_start(lvT[:, :], lv_sb[:, 0, :])

        mu_sb = dense(h1, "mu", AF.Identity, F32)
        nc.sync.dma_start(muT[:, :], mu_sb[:, 0, :])

        # z = mu + eps * exp(0.5 lv)
        eps_sb = data.tile([P, cap], F32)
        nc.sync.dma_start(eps_sb[:], epsT[:, :])
        z_sb = data.tile([P, 1, cap], F32R)
        nc.vector.scalar_tensor_tensor(
            z_sb[:, 0, :],
            eps_sb[:],
            1.0,
            t_sb[:],
            mybir.AluOpType.mult,
            mybir.AluOpType.mult,
        )
        nc.vector.tensor_add(z_sb[:, 0, :], z_sb[:, 0, :], mu_sb[:, 0, :])

        # decoder
        d0 = dense(z_sb, "dec0", AF.Relu, F32R, out_tag="a_enc1")
        d1 = dense(d0, "dec1", AF.Relu, F32R, out_tag="a_enc0")

        # output layer: stage through SBUF, DMA out per (o, n)
        def out_evac(o, n0, nb, ps, b_ap):
            st = stage.tile([P, 512], F32, tag="ostage", name=f"out_st{o}")
            nc.scalar.activation(st[:, :nb], ps[:, :nb], AF.Identity, bias=b_ap)
            nc.sync.dma_start(reconT[o * P : (o + 1) * P, n0 : n0 + nb], st[:, :nb])

        dense(d1, "out", AF.Identity, F32, evac=out_evac)

    nc.compile()
    return nc


def _get_program(cap):
    if cap not in _program_cache:
        _program_cache[cap] = _build(cap)
    return _program_cache[cap]


def _pack_w(w):
    """[K, O] -> [OT, 128, KT, 128] contiguous per-o-tile blocks."""
    K, O = w.shape
    return np.ascontiguousarray(
        w.reshape(K // P, P, O // P, P).transpose(2, 1, 0, 3)
    )


def _pack_b(b):
    """[O] -> [128, OT]"""
    return np.ascontiguousarray(b.reshape(-1, P).T)


def _route(cluster_labels):
    labels = np.asarray(cluster_labels).astype(np.int64)
    idx = [np.nonzero(labels == c)[0] for c in range(C)]
    cap = max(256, max((len(i) for i in idx), default=1))
    cap = (cap + 3) // 4 * 4
    idx_pad = []
    for c in range(C):
        i = idx[c]
        if len(i) == 0:
            pad = np.zeros(cap, dtype=np.int64)
        else:
            pad = np.concatenate([i, np.full(cap - len(i), i[0], dtype=np.int64)])
        idx_pad.append(pad)
    return idx, idx_pad, cap


def _run(inputs, trace=False):
    x = np.ascontiguousarray(np.asarray(inputs["x"], dtype=np.float32))
    eps = np.ascontiguousarray(np.asarray(inputs["eps"], dtype=np.float32))
    idx, idx_pad, cap = _route(inputs["cluster_labels"])
    nc = _get_program(cap)

    f32 = lambda a: np.asarray(a, dtype=np.float32)
    shared_w = {
        "w_enc1": _pack_w(f32(inputs["W_enc1"])),
        "w_lv": _pack_w(f32(inputs["W_logvar"])),
        "w_mu": _pack_w(f32(inputs["W_mu"])),
        "w_dec0": _pack_w(f32(inputs["W_dec0"])),
        "w_out": _pack_w(f32(inputs["W_out"])),
    }
    b_shared = {
        "enc1": _pack_b(f32(inputs["b_enc1"])),
        "lv": _pack_b(f32(inputs["b_logvar"])),
        "mu": _pack_b(f32(inputs["b_mu"])),
        "dec0": _pack_b(f32(inputs["b_dec0"])),
        "out": _pack_b(f32(inputs["b_out"])),
    }
    in_maps = []
    for c in range(C):
        ip = idx_pad[c]
        xT = x[ip].T  # [D_IN, cap]
        bias_all = np.concatenate(
            [
                _pack_b(f32(inputs["b_enc0"][c])),
                b_shared["enc1"],
                b_shared["lv"],
                b_shared["mu"],
                b_shared["dec0"],
                _pack_b(f32(inputs["b_dec1"][c])),
                b_shared["out"],
            ],
            axis=1,
        )
        in_maps.append(
            {
                "xh": np.ascontiguousarray(
                    xT.reshape(D_IN // P, P, cap).transpose(1, 0, 2)
                ),
                "epsT": np.ascontiguousarray(eps[ip].T),
                "w_enc0": _pack_w(f32(inputs["W_enc0"][c])),
                "w_dec1": _pack_w(f32(inputs["W_dec1"][c])),
                "bias_all": np.ascontiguousarray(bias_all),
                **shared_w,
            }
        )

    res = run_bass_kernel_spmd(nc, in_maps, core_ids=list(range(N_CORES)), trace=trace)

    B = x.shape[0]
    recon = np.empty((B, D_IN), dtype=np.float32)
    mu = np.empty((B, LAT), dtype=np.float32)
    logvar = np.empty((B, LAT), dtype=np.float32)
    for c in range(C):
        i = idx[c]
        if len(i) == 0:
            continue
        r = res.results[c]
        recon[i] = r["reconT"][:, : len(i)].T
        mu[i] = r["muT"][:, : len(i)].T
        logvar[i] = r["lvT"][:, : len(i)].T
    return (recon, mu, logvar), res


def kernel(**inputs):
    outs, _ = _run(inputs, trace=False)
    return outs


# revision 18
# speedup vs baseline: 1.3464x; 1.0433x over previous
"""CISSVAE (cluster-routed VAE) Trainium2 kernel.

Strategy: expert-parallel over the 8 clusters — core c handles exactly the rows
with cluster_labels == c (capacity-padded to a fixed CAP so all 8 cores run one
SPMD program). Host does the routing (gather by cluster, pad, transpose to
feature-major) and the inverse scatter. On-device everything is dense matmuls:

    h0 = relu(W_enc0[c]^T x + b)   [1024, CAP]
    h1 = relu(W_enc1^T h0 + b)     [512, CAP]
    lv = W_logvar^T h1 + b         [128, CAP]
    mu = W_mu^T h1 + b             [128, CAP]
    z  = mu + eps * exp(0.5 lv)    [128, CAP]
    d0 = relu(W_dec0^T z + b)      [512, CAP]
    d1 = relu(W_dec1[c]^T d0 + b)  [1024, CAP]
    recon = W_out^T d1 + b         [2048, CAP]

Activations live feature-major ([feature, column]) so every layer is
out[o, n] += W[k, o] * act[k, n] with W slices as the stationary operand —
no transposes anywhere on device. Matmuls run in float32r (TF32-like, full
PE rate for moving dim >= 256), accumulation in fp32 PSUM, bias+activation
fused into the ScalarE PSUM evacuation. All weights are host-packed into
[OT, 128, KT, 128] per-o-tile blocks so every weight DMA is contiguous.
"""

import math
from contextlib import ExitStack

import numpy as np

import concourse.bacc as bacc
import concourse.mybir as mybir
import concourse.tile as tile
from concourse.bass_utils import run_bass_kernel_spmd

F32 = mybir.dt.float32
F32R = mybir.dt.float32r
AF = mybir.ActivationFunctionType

D_IN, H0, H1, LAT, C = 2048, 1024, 512, 128, 8
N_CORES = 8
P = 128

# (name, K, O) for the seven dense layers, in execution order
LAYERS = [
    ("enc0", D_IN, H0),
    ("enc1", H0, H1),
    ("lv", H1, LAT),
    ("mu", H1, LAT),
    ("dec0", LAT, H1),
    ("dec1", H1, H0),
    ("out", H0, D_IN),
]
BIAS_COLS = sum(o // P for _, _, o in LAYERS)  # 42

_program_cache: dict = {}


def _nchunks(cap):
    """Split cap columns into balanced chunks, each in [256, 512] and a
    multiple of 4 (fp32r matmul ISA restriction on moving dim/offset)."""
    assert cap % 4 == 0
    k = max(1, math.ceil(cap / 512))
    if k > 1 and cap / k < 256:
        k -= 1
    q = cap // 4
    base = q // k
    rem = q - base * k
    sizes = [4 * (base + (1 if i < rem else 0)) for i in range(k)]
    assert all(256 <= s <= 512 for s in sizes) or cap < 256, (cap, sizes)
    out, acc = [], 0
    for s in sizes:
        out.append((acc, s))
        acc += s
    return out


def _build(cap):
    nbs = _nchunks(cap)
    nc = bacc.Bacc(trn_type="TRN2", target_bir_lowering=False, debug=False)

    KT_X = D_IN // P
    xh = nc.dram_tensor("xh", [P, KT_X, cap], F32R, kind="ExternalInput")
    epsT = nc.dram_tensor("epsT", [LAT, cap], F32, kind="ExternalInput")
    w_d = {
        name: nc.dram_tensor(f"w_{name}", [o // P, P, k // P, P], F32R,
                             kind="ExternalInput")
        for name, k, o in LAYERS
    }
    bias_d = nc.dram_tensor("bias_all", [P, BIAS_COLS], F32, kind="ExternalInput")

    reconT = nc.dram_tensor("reconT", [D_IN, cap], F32, kind="ExternalOutput")
    muT = nc.dram_tensor("muT", [LAT, cap], F32, kind="ExternalOutput")
    lvT = nc.dram_tensor("lvT", [LAT, cap], F32, kind="ExternalOutput")

    bias_off = {}
    acc = 0
    for name, _, o in LAYERS:
        bias_off[name] = acc
        acc += o // P

    with tile.TileContext(nc) as tc, ExitStack() as ctx:
        data = ctx.enter_context(tc.tile_pool(name="data", bufs=1))
        wstream = ctx.enter_context(tc.tile_pool(name="wstream", bufs=4))
        stage = ctx.enter_context(tc.tile_pool(name="stage", bufs=4))
        psum = ctx.enter_context(tc.tile_pool(name="psum", bufs=8, space="PSUM"))

        # enc0's first weight slice goes first so it heads its DMA queue:
        # the very first matmul then only waits on it + x[0]. Further enc0
        # slices interleave with the x chunks so o=1..3 never stall PE.
        prefetched = {}

        def prefetch_w(lname, o):
            wt = wstream.tile([P, 16, P], F32R, tag="wstream", name=f"{lname}_w{o}p")
            nc.sync.dma_start(wt[:, :, :], w_d[lname][o, :, :, :])
            prefetched[(lname, o)] = wt

        prefetch_w("enc0", 0)

        # x: per-k-tile contiguous DMAs (parallel queues, fine-grained deps)
        xt = data.tile([P, KT_X, cap], F32R, tag="slabA")
        for k in range(KT_X):
            nc.sync.dma_start(xt[:, k, :], xh[:, k, :])
            if k in (3, 7, 11):
                prefetch_w("enc0", (k + 1) // 4)

        bias_sb = data.tile([P, BIAS_COLS], F32)
        nc.sync.dma_start(bias_sb[:], bias_d[:, :])

        # small mid-layer weights resident, prefetched up front so the
        # lv->mu->z->dec0 section never waits on DMA
        def load_resident(lname):
            _, K, O = next(l for l in LAYERS if l[0] == lname)
            KT = K // P
            wt = data.tile([P, KT, O // P, P], F32R, name=f"wres_{lname}")
            nc.sync.dma_start(
                wt[:], w_d[lname].rearrange("ot kp kt of -> kp kt ot of")
            )
            return wt

        w_res = {n: load_resident(n) for n in ("lv", "mu", "dec0")}

        def dense(inp, lname, act, out_dt, out_tag=None, evac=None):
            """One dense layer; weights streamed per 128-wide o-slice from the
            packed DRAM block. inp: SBUF [P, KT, cap]. Returns [P, OT, cap]
            (unless evac is given, which handles PSUM evacuation itself)."""
            _, K, O = next(l for l in LAYERS if l[0] == lname)
            KT, OT = K // P, O // P
            out = None
            if evac is None:
                out = data.tile(
                    [P, OT, cap], out_dt, name=f"a_{lname}", tag=out_tag or f"a_{lname}"
                )
            for o in range(OT):
                if lname in w_res:
                    w_ap = lambda k, o=o: w_res[lname][:, k, o, :]
                elif (lname, o) in prefetched:
                    w_ap = lambda k, t=prefetched[lname, o]: t[:, k, :]
                else:
                    wt = wstream.tile(
                        [P, 16, P], F32R, tag="wstream", name=f"{lname}_w{o}"
                    )
                    nc.sync.dma_start(wt[:, :KT, :], w_d[lname][o, :, :, :])
                    w_ap = lambda k, wt=wt: wt[:, k, :]
                for n0, nb in nbs:
                    ps = psum.tile([P, 512], F32, tag="ps", name=f"{lname}_ps{o}")
                    for k in range(KT):
                        nc.tensor.matmul(
                            ps[:, :nb],
                            w_ap(k),
                            inp[:, k, n0 : n0 + nb],
                            start=(k == 0),
                            stop=(k == KT - 1),
                        )
                    b_ap = bias_sb[:, bias_off[lname] + o : bias_off[lname] + o + 1]
                    if evac is None:
                        nc.scalar.activation(
                            out[:, o, n0 : n0 + nb], ps[:, :nb], act, bias=b_ap
                        )
                    else:
                        evac(o, n0, nb, ps, b_ap)
            return out

        # encoder
        h0 = dense(xt, "enc0", AF.Relu, F32R)
        h1 = dense(h0, "enc1", AF.Relu, F32R)

        # latent heads: lv first so the exp/z chain overlaps the mu matmuls
        eps_sb = data.tile([P, cap], F32)
        nc.sync.dma_start(eps_sb[:], epsT[:, :])

        lv_sb = dense(h1, "lv", AF.Identity, F32)
        t_sb = data.tile([P, cap], F32)
        for n0, nb in nbs:
            nc.scalar.activation(
                t_sb[:, n0 : n0 + nb], lv_sb[:, 0, n0 : n0 + nb], AF.Exp, scale=0.5
            )
        nc.sync.dma_start(lvT[:, :], lv_sb[:, 0, :])

        mu_sb = dense(h1, "mu", AF.Identity, F32)
        nc.sync.dma_start(muT[:, :], mu_sb[:, 0, :])

        # z = mu + eps * exp(0.5 lv), chunked so dec0 can start on chunk 0
        z_sb = data.tile([P, 1, cap], F32R)
        for n0, nb in nbs:
            sl = slice(n0, n0 + nb)
            nc.vector.scalar_tensor_tensor(
                z_sb[:, 0, sl],
                eps_sb[:, sl],
                1.0,
                t_sb[:, sl],
                mybir.AluOpType.mult,
                mybir.AluOpType.mult,
            )
            nc.vector.tensor_add(z_sb[:, 0, sl], z_sb[:, 0, sl], mu_sb[:, 0, sl])

        # decoder
        d0 = dense(z_sb, "dec0", AF.Relu, F32R, out_tag="a_enc1")
        d1 = dense(d0, "dec1", AF.Relu, F32R, out_tag="a_enc0")

        # output layer: stage through SBUF, DMA out per (o, n)
        def out_evac(o, n0, nb, ps, b_ap):
            st = stage.tile([P, 512], F32, tag="ostage", name=f"out_st{o}")
            nc.scalar.activation(st[:, :nb], ps[:, :nb], AF.Identity, bias=b_ap)
            nc.sync.dma_start(reconT[o * P : (o + 1) * P, n0 : n0 + nb], st[:, :nb])

        dense(d1, "out", AF.Identity, F32, evac=out_evac)

    nc.compile()
    return nc


def _get_program(cap):
    if cap not in _program_cache:
        _program_cache[cap] = _build(cap)
    return _program_cache[cap]


def _pack_w(w):
    """[K, O] -> [OT, 128, KT, 128] contiguous per-o-tile blocks."""
    K, O = w.shape
    return np.ascontiguousarray(
        w.reshape(K // P, P, O // P, P).transpose(2, 1, 0, 3)
    )


def _pack_b(b):
    """[O] -> [128, OT]"""
    return np.ascontiguousarray(b.reshape(-1, P).T)


def _route(cluster_labels):
    labels = np.asarray(cluster_labels).astype(np.int64)
    idx = [np.nonzero(labels == c)[0] for c in range(C)]
    cap = max(256, max((len(i) for i in idx), default=1))
    cap = (cap + 3) // 4 * 4
    idx_pad = []
    for c in range(C):
        i = idx[c]
        if len(i) == 0:
            pad = np.zeros(cap, dtype=np.int64)
        else:
            pad = np.concatenate([i, np.full(cap - len(i), i[0], dtype=np.int64)])
        idx_pad.append(pad)
    return idx, idx_pad, cap


def _run(inputs, trace=False):
    x = np.ascontiguousarray(np.asarray(inputs["x"], dtype=np.float32))
    eps = np.ascontiguousarray(np.asarray(inputs["eps"], dtype=np.float32))
    idx, idx_pad, cap = _route(inputs["cluster_labels"])
    nc = _get_program(cap)

    f32 = lambda a: np.asarray(a, dtype=np.float32)
    shared_w = {
        "w_enc1": _pack_w(f32(inputs["W_enc1"])),
        "w_lv": _pack_w(f32(inputs["W_logvar"])),
        "w_mu": _pack_w(f32(inputs["W_mu"])),
        "w_dec0": _pack_w(f32(inputs["W_dec0"])),
        "w_out": _pack_w(f32(inputs["W_out"])),
    }
    b_shared = {
        "enc1": _pack_b(f32(inputs["b_enc1"])),
        "lv": _pack_b(f32(inputs["b_logvar"])),
        "mu": _pack_b(f32(inputs["b_mu"])),
        "dec0": _pack_b(f32(inputs["b_dec0"])),
        "out": _pack_b(f32(inputs["b_out"])),
    }
    in_maps = []
    for c in range(C):
        ip = idx_pad[c]
        xT = x[ip].T  # [D_IN, cap]
        bias_all = np.concatenate(
            [
                _pack_b(f32(inputs["b_enc0"][c])),
                b_shared["enc1"],
                b_shared["lv"],
                b_shared["mu"],
                b_shared["dec0"],
                _pack_b(f32(inputs["b_dec1"][c])),
                b_shared["out"],
            ],
            axis=1,
        )
        in_maps.append(
            {
                "xh": np.ascontiguousarray(
                    xT.reshape(D_IN // P, P, cap).transpose(1, 0, 2)
                ),
                "epsT": np.ascontiguousarray(eps[ip].T),
                "w_enc0": _pack_w(f32(inputs["W_enc0"][c])),
                "w_dec1": _pack_w(f32(inputs["W_dec1"][c])),
                "bias_all": np.ascontiguousarray(bias_all),
                **shared_w,
            }
        )

    res = run_bass_kernel_spmd(nc, in_maps, core_ids=list(range(N_CORES)), trace=trace)

    B = x.shape[0]
    recon = np.empty((B, D_IN), dtype=np.float32)
    mu = np.empty((B, LAT), dtype=np.float32)
    logvar = np.empty((B, LAT), dtype=np.float32)
    for c in range(C):
        i = idx[c]
        if len(i) == 0:
            continue
        r = res.results[c]
        recon[i] = r["reconT"][:, : len(i)].T
        mu[i] = r["muT"][:, : len(i)].T
        logvar[i] = r["lvT"][:, : len(i)].T
    return (recon, mu, logvar), res


def kernel(**inputs):
    outs, _ = _run(inputs, trace=False)
    return outs
